# revision 1
# baseline (speedup 1.0000x reference)
"""Low-rank attention Trainium2 kernel (8 NeuronCores, SPMD).

Math (reference):
    tmp = relu(x @ W.T + b); U,V,Z,T = split(tmp, 4, axis=1)
    norm = sum(U @ colsum(V)) / n + eps ;  D = 1/norm
    out = concat[(U @ (V.T @ Z)) * D, T]

Sharding: rows of x across 8 cores. Per-core partials (V.T@Z [k,k],
colsum(V), colsum(U)) are AllReduced on-device; each core then computes
its local U @ (VtZ) * D.

Layout trick: x is passed pre-transposed per shard (xT [d, n_loc]) so both
matmul orientations stream straight from HBM with d on partitions.
float32r matmul dtype: full fp32 storage, ~1e-4 matmul rounding, 1 cyc/row.
"""
import sys

sys.path.insert(0, "/opt/trn_rl_repo")
import numpy as np

NCORES = 8
N_ROWS, D_IN, K = 65536, 1024, 256
NLOC = N_ROWS // NCORES      # 8192 rows per core
P = 128
IB = 512                     # i-block width
NB = NLOC // IB              # 16 blocks
EPS = 1e-6
TDEF = 8                     # T-pass blocks deferred to overlap the AllReduce

_built = {}


def _build(d_rows):
    import concourse.bacc as bacc
    import concourse.mybir as mybir
    import concourse.tile as tile

    dt = mybir.dt
    f32, f32r = dt.float32, dt.float32r
    RELU = mybir.ActivationFunctionType.Relu
    DT = d_rows // P
    NSUB = IB // P

    nc = bacc.Bacc("TRN2", target_bir_lowering=False, debug=False, num_devices=NCORES)
    xT = nc.dram_tensor("xT", [d_rows, NLOC], f32r, kind="ExternalInput")
    WT = nc.dram_tensor("WT", [d_rows, 4 * K], f32r, kind="ExternalInput")
    onesc = nc.dram_tensor("onesc", [P, 1], f32r, kind="ExternalInput")
    out = nc.dram_tensor("out", [NLOC, 2 * K], f32, kind="ExternalOutput")

    with tile.TileContext(nc) as tc:
        with (
            tc.tile_pool(name="wp", bufs=1) as wp,
            tc.tile_pool(name="xp", bufs=4) as xp,
            tc.tile_pool(name="up", bufs=1) as up,
            tc.tile_pool(name="vzp", bufs=6) as vzp,
            tc.tile_pool(name="op", bufs=6) as op,
            tc.tile_pool(name="acc", bufs=1) as accp,
            tc.tile_pool(name="ps", bufs=6, space="PSUM") as ps,
            tc.tile_pool(name="dram", bufs=1, space="DRAM") as dram,
        ):
            wt = []
            for kd in range(DT):
                w = wp.tile([P, 4 * K], f32r, tag=f"w{kd}", name=f"w{kd}")
                nc.gpsimd.dma_start(out=w[:], in_=WT[kd * P:(kd + 1) * P, :])
                wt.append(w)
            ones_r = wp.tile([P, 1], f32r, tag="ones_r")
            nc.sync.dma_start(out=ones_r[:], in_=onesc[:, :])
            ones_row = wp.tile([1, P], f32, tag="ones_row")
            nc.vector.memset(ones_row[:], 1.0)

            ut = [up.tile([P, NLOC], f32r, tag=f"ut{h}", name=f"ut{h}") for h in range(2)]
            csu_cols = [accp.tile([P, NB], f32, tag=f"csuc{h}", name=f"csuc{h}") for h in range(2)]
            vtz_acc = [accp.tile([P, K], f32, tag=f"vtza{h}", name=f"vtza{h}") for h in range(2)]
            csv_acc = accp.tile([1, K], f32, tag="csva")

            # ---- phase 1: projection + partial reductions ----
            for ib in range(NB):
                xt = []
                for kd in range(DT):
                    t = xp.tile([P, IB], f32r, tag=f"x{kd}", name=f"x{kd}")
                    nc.sync.dma_start(
                        out=t[:], in_=xT[kd * P:(kd + 1) * P, ib * IB:(ib + 1) * IB]
                    )
                    xt.append(t)
                # U^T [k1, i] — stationary Wu^T, moving x^T; relu on ACT with
                # free-dim running sum (colsum_U partial) via accum_out.
                for h in range(2):
                    pu = ps.tile([P, IB], f32, tag="work")
                    for kd in range(DT):
                        nc.tensor.matmul(
                            pu[:], wt[kd][:, h * P:(h + 1) * P], xt[kd][:],
                            start=(kd == 0), stop=(kd == DT - 1),
                        )
                    nc.scalar.activation(
                        ut[h][:, ib * IB:(ib + 1) * IB], pu[:], RELU,
                        accum_out=csu_cols[h][:, ib:ib + 1],
                    )
                # V|Z and T in natural [i, j] layout per 128-row subtile
                vz_tiles = []
                for s in range(NSUB):
                    i0 = ib * IB + s * P
                    pvz = ps.tile([P, IB], f32, tag="work")
                    for kd in range(DT):
                        nc.tensor.matmul(
                            pvz[:], xt[kd][:, s * P:(s + 1) * P], wt[kd][:, K:3 * K],
                            start=(kd == 0), stop=(kd == DT - 1),
                        )
                    vz = vzp.tile([P, 2 * K], f32r, tag="vz")
                    nc.vector.tensor_relu(vz[:], pvz[:])
                    vz_tiles.append(vz)
                    if ib < NB - TDEF:
                        pt = ps.tile([P, K], f32, tag="work")
                        for kd in range(DT):
                            nc.tensor.matmul(
                                pt[:], xt[kd][:, s * P:(s + 1) * P], wt[kd][:, 3 * K:4 * K],
                                start=(kd == 0), stop=(kd == DT - 1),
                            )
                        ot = op.tile([P, K], f32, tag="ot")
                        nc.vector.tensor_relu(ot[:], pt[:])
                        nc.sync.dma_start(out=out[i0:i0 + P, K:2 * K], in_=ot[:])
                # VtZ partial: contract i (partitions) over this block
                for h in range(2):
                    pz = ps.tile([P, K], f32, tag="work")
                    for s in range(NSUB):
                        nc.tensor.matmul(
                            pz[:], vz_tiles[s][:, h * P:(h + 1) * P],
                            vz_tiles[s][:, K:2 * K],
                            start=(s == 0), stop=(s == NSUB - 1),
                        )
                    if ib == 0:
                        nc.vector.tensor_copy(vtz_acc[h][:], pz[:])
                    else:
                        nc.vector.tensor_add(vtz_acc[h][:], vtz_acc[h][:], pz[:])
                # colsum_V partial via ones-matmul
                pcs = ps.tile([1, K], f32, tag="work")
                for s in range(NSUB):
                    nc.tensor.matmul(
                        pcs[:], ones_r[:], vz_tiles[s][:, 0:K],
                        start=(s == 0), stop=(s == NSUB - 1),
                    )
                if ib == 0:
                    nc.vector.tensor_copy(csv_acc[:], pcs[:])
                else:
                    nc.vector.tensor_add(csv_acc[:], csv_acc[:], pcs[:])

            # ---- phase 2: AllReduce the [k,k]+[k]+[k] partials ----
            csu = [accp.tile([P, 1], f32, tag=f"csu{h}", name=f"csu{h}") for h in range(2)]
            for h in range(2):
                nc.vector.reduce_sum(csu[h][:], csu_cols[h][:], axis=mybir.AxisListType.X)
            bin_ = dram.tile([2 * P + 3, K], f32)
            bout = dram.tile([2 * P + 3, K], f32)
            for h in range(2):
                nc.sync.dma_start(out=bin_[h * P:(h + 1) * P, :], in_=vtz_acc[h][:])
            nc.sync.dma_start(out=bin_[2 * P:2 * P + 1, :], in_=csv_acc[:])
            for h in range(2):
                nc.sync.dma_start(
                    out=bin_[2 * P + 1 + h, 0:P].rearrange("(p one) -> p one", one=1),
                    in_=csu[h][:],
                )
            nc.gpsimd.collective_compute(
                "AllReduce", mybir.AluOpType.add,
                replica_groups=[list(range(NCORES))],
                ins=[bin_.opt()], outs=[bout.opt()],
            )
            # ---- deferred T-pass: keeps PE busy/warm during the AllReduce ----
            for ib in range(NB - TDEF, NB):
                xt = []
                for kd in range(DT):
                    t = xp.tile([P, IB], f32r, tag=f"x{kd}", name=f"xd{kd}")
                    nc.sync.dma_start(
                        out=t[:], in_=xT[kd * P:(kd + 1) * P, ib * IB:(ib + 1) * IB]
                    )
                    xt.append(t)
                for s in range(NSUB):
                    i0 = ib * IB + s * P
                    pt = ps.tile([P, K], f32, tag="work")
                    for kd in range(DT):
                        nc.tensor.matmul(
                            pt[:], xt[kd][:, s * P:(s + 1) * P], wt[kd][:, 3 * K:4 * K],
                            start=(kd == 0), stop=(kd == DT - 1),
                        )
                    ot = op.tile([P, K], f32, tag="ot")
                    nc.vector.tensor_relu(ot[:], pt[:])
                    nc.sync.dma_start(out=out[i0:i0 + P, K:2 * K], in_=ot[:])

            # ---- phase 3: D = 1/(csU.csV/n + eps); scale VtZ ----
            vtzf = [accp.tile([P, K], f32, tag=f"vtzf{h}", name=f"vtzf{h}") for h in range(2)]
            for h in range(2):
                nc.sync.dma_start(out=vtzf[h][:], in_=bout[h * P:(h + 1) * P, :])
            csvt = accp.tile([P, 2], f32, tag="csvt")
            nc.sync.dma_start(out=csvt[:], in_=bout[2 * P, :].rearrange("(t p) -> p t", p=P))
            csut = accp.tile([P, 2], f32, tag="csut")
            nc.sync.dma_start(
                out=csut[:], in_=bout[2 * P + 1:2 * P + 3, 0:P].rearrange("t p -> p t")
            )
            pdot = ps.tile([1, 1], f32, tag="work")
            for h in range(2):
                nc.tensor.matmul(
                    pdot[:], csut[:, h:h + 1], csvt[:, h:h + 1],
                    start=(h == 0), stop=(h == 1),
                )
            dsb = accp.tile([1, 1], f32, tag="dsb")
            nc.vector.tensor_scalar(
                out=dsb[:], in0=pdot[:], scalar1=1.0 / N_ROWS, scalar2=EPS,
                op0=mybir.AluOpType.mult, op1=mybir.AluOpType.add,
            )
            nc.vector.reciprocal(dsb[:], dsb[:])
            pb = ps.tile([P, 1], f32, tag="work")
            nc.tensor.matmul(pb[:], ones_row[:], dsb[:], start=True, stop=True)
            dbc = accp.tile([P, 1], f32, tag="dbc")
            nc.vector.tensor_copy(dbc[:], pb[:])
            vtzr = [accp.tile([P, K], f32r, tag=f"vtzr{h}", name=f"vtzr{h}") for h in range(2)]
            for h in range(2):
                nc.vector.tensor_scalar_mul(vtzr[h][:], vtzf[h][:], dbc[:])

            # ---- phase 4: res = U @ (VtZ * D), written row-natural ----
            for ib in range(NB):
                for s in range(NSUB):
                    i0 = ib * IB + s * P
                    pr = ps.tile([P, K], f32, tag="work")
                    for h in range(2):
                        nc.tensor.matmul(
                            pr[:], ut[h][:, i0:i0 + P], vtzr[h][:],
                            start=(h == 0), stop=(h == 1),
                        )
                    orow = op.tile([P, K], f32, tag="ot")
                    nc.vector.tensor_copy(orow[:], pr[:])
                    nc.sync.dma_start(out=out[i0:i0 + P, 0:K], in_=orow[:])

    nc.compile()
    return nc


def _get_nc(d_rows):
    if d_rows not in _built:
        _built[d_rows] = _build(d_rows)
    return _built[d_rows]


def _run(x, W, b, trace=False, trace_cores=None):
    from concourse.bass_utils import run_bass_kernel_spmd

    x = np.ascontiguousarray(x, dtype=np.float32)
    W = np.ascontiguousarray(W, dtype=np.float32)
    b = np.asarray(b, dtype=np.float32)
    if np.any(b):
        d_rows = 1152  # pad contraction: extra ones-row in x picks up b from W
        WT_full = np.zeros((d_rows, 4 * K), np.float32)
        WT_full[:D_IN] = W.T
        WT_full[D_IN] = b
    else:
        d_rows = D_IN
        WT_full = np.ascontiguousarray(W.T)
    nc = _get_nc(d_rows)
    in_maps = []
    for c in range(NCORES):
        xs = x[c * NLOC:(c + 1) * NLOC]
        if d_rows == D_IN:
            xTs = np.ascontiguousarray(xs.T)
        else:
            xTs = np.zeros((d_rows, NLOC), np.float32)
            xTs[:D_IN] = xs.T
            xTs[D_IN] = 1.0
        in_maps.append({"xT": xTs, "WT": WT_full, "onesc": np.ones((P, 1), np.float32)})
    res = run_bass_kernel_spmd(
        nc, in_maps, list(range(NCORES)),
        trace=trace, **({"trace_cores": trace_cores} if trace_cores else {}),
    )
    full = np.concatenate([res.results[c]["out"] for c in range(NCORES)], axis=0)
    return full, res


def kernel(x, W, b):
    full, _ = _run(x, W, b)
    return full



# revision 5
# speedup vs baseline: 1.3763x; 1.3763x over previous
"""Low-rank attention Trainium2 kernel (8 NeuronCores, SPMD) — fp8 DoubleRow.

Math (reference):
    tmp = relu(x @ W.T + b); U,V,Z,T = split(tmp, 4, axis=1)
    norm = sum(U @ colsum(V)) / n + eps ;  D = 1/norm
    out = concat[(U @ (V.T @ Z)) * D, T]

Sharding: rows of x across 8 cores. Per-core partials (V.T@Z [k,k],
colsum(V), colsum(U)) are AllReduced on-device; each core then computes
its local U @ (VtZ) * D.

Precision strategy (rel-err budget 2e-2; this lands ~4e-3):
  - U/V/Z projection, VtZ, colsums and U@(VtZ) run in fp8e4m3 with the
    tensor engine's DoubleRow perf mode (2 fp8 MACs/PE/cycle, paired
    256-deep contraction per instruction). Elementwise fp8 noise washes
    out in the n=65536 (VtZ/colsum) and k=256 (U@VtZ) reductions.
  - T passthrough is bf16 (error shows up directly in the output).
  - W_uvz is pre-scaled by SW=16 to clear the fp8 subnormal range; all
    scales are folded into the final copies (exact powers of two).

Layouts: x and W are host-packed as [128, DJ, n] so each DoubleRow
matmul slices contraction pairs straight from SBUF. T and res are
computed transposed ([col, row]) so every matmul keeps a 512-wide
moving dim; the host transposes them back during the gather.
"""
import sys

sys.path.insert(0, "/opt/trn_rl_repo")
import numpy as np

NCORES = 8
N_ROWS, D_IN, K = 65536, 1024, 256
NLOC = N_ROWS // NCORES      # 8192 rows per core
P = 128
IB = 512                     # i-block width
NB = NLOC // IB              # 16 blocks
EPS = 1e-6
SW = 16.0                    # fp8 weight pre-scale
S4 = 256.0                   # VtZ*D quantization scale

_built = {}


def _build(dj):
    """dj = number of 128-row contraction chunks (8 normally, 10 with bias pad)."""
    import concourse.bacc as bacc
    import concourse.mybir as mybir
    import concourse.tile as tile

    dt = mybir.dt
    f32, f8, bf16 = dt.float32, dt.float8e4, dt.bfloat16
    RELU = mybir.ActivationFunctionType.Relu
    COPY = mybir.ActivationFunctionType.Copy
    DR = mybir.MatmulPerfMode.DoubleRow
    NJP = dj // 2            # DoubleRow contraction pair count

    nc = bacc.Bacc("TRN2", target_bir_lowering=False, debug=False, num_devices=NCORES)
    x8d = nc.dram_tensor("x8", [P, dj, NLOC], f8, kind="ExternalInput")
    xbd = nc.dram_tensor("xb", [P, dj, NLOC], bf16, kind="ExternalInput")
    w8d = nc.dram_tensor("w8", [P, dj, 3 * K], f8, kind="ExternalInput")
    wtd = nc.dram_tensor("wt", [P, dj, K], bf16, kind="ExternalInput")
    outR = nc.dram_tensor("outR", [K, NLOC], f32, kind="ExternalOutput")
    outT = nc.dram_tensor("outT", [K, NLOC], f32, kind="ExternalOutput")

    with tile.TileContext(nc) as tc:
        with (
            tc.tile_pool(name="wp", bufs=1) as wp,
            tc.tile_pool(name="xp", bufs=3) as xp,
            tc.tile_pool(name="xbp", bufs=2) as xbp,
            tc.tile_pool(name="up", bufs=1) as up,
            tc.tile_pool(name="vzp", bufs=2) as vzp,
            tc.tile_pool(name="op", bufs=4) as op,
            tc.tile_pool(name="acc", bufs=1) as accp,
            tc.tile_pool(name="ps", bufs=6, space="PSUM") as ps,
            tc.tile_pool(name="vps", bufs=2, space="PSUM") as vps,
            tc.tile_pool(name="dram", bufs=1, space="DRAM") as dram,
        ):
            w8t = wp.tile([P, dj, 3 * K], f8, tag="w8t")
            nc.gpsimd.dma_start(out=w8t[:], in_=w8d[:, :, :])
            wtt = wp.tile([P, dj, K], bf16, tag="wtt")
            nc.gpsimd.dma_start(out=wtt[:], in_=wtd[:, :, :])
            ones_row = wp.tile([1, P], f32, tag="ones_row")
            nc.vector.memset(ones_row[:], 1.0)

            ut = up.tile([P, 2, NLOC], f8, tag="ut")
            csu_cols = [accp.tile([P, NB], f32, tag=f"csuc{h}", name=f"csuc{h}")
                        for h in range(2)]
            vtz_acc = [accp.tile([P, K + 1], f32, tag=f"vtza{h}", name=f"vtza{h}")
                       for h in range(2)]

            # ---- phase A: fp8 projection (U^T, V|Z) + VtZ/colsum partials ----
            for ib in range(NB):
                i0 = ib * IB
                x8t = xp.tile([P, dj, IB], f8, tag="x8t")
                nc.sync.dma_start(out=x8t[:], in_=x8d[:, :, i0:i0 + IB])
                # U^T [k-chunk, i]: stationary W pairs, moving x pairs
                for h in range(2):
                    pu = ps.tile([P, IB], f32, tag="work")
                    for jp in range(NJP):
                        nc.tensor.matmul(
                            pu[:], w8t[:, 2 * jp:2 * jp + 2, h * P:(h + 1) * P],
                            x8t[:, 2 * jp:2 * jp + 2, :],
                            start=(jp == 0), stop=(jp == NJP - 1), perf_mode=DR,
                        )
                    nc.scalar.activation(
                        ut[:, h, i0:i0 + IB], pu[:], RELU,
                        accum_out=csu_cols[h][:, ib:ib + 1],
                    )
                # V|Z natural [i, col] per 128-row subtile; +1 ones column
                vzt = vzp.tile([P, 4, 3 * K], f8, tag="vzt")
                nc.vector.memset(vzt[:, :, 2 * K:2 * K + 1], 1.0)
                for s in range(4):
                    pvz = ps.tile([P, 2 * K], f32, tag="work")
                    for jp in range(NJP):
                        nc.tensor.matmul(
                            pvz[:], x8t[:, 2 * jp:2 * jp + 2, s * P:(s + 1) * P],
                            w8t[:, 2 * jp:2 * jp + 2, K:3 * K],
                            start=(jp == 0), stop=(jp == NJP - 1), perf_mode=DR,
                        )
                    nc.vector.tensor_relu(vzt[:, s, 0:2 * K], pvz[:])
                # VtZ partial + colsum(V) via the ones column: V^T @ [Z | 1]
                for h in range(2):
                    pz = vps.tile([P, K + 1], f32, tag="vtzw")
                    for g in range(2):
                        nc.tensor.matmul(
                            pz[:], vzt[:, 2 * g:2 * g + 2, h * P:(h + 1) * P],
                            vzt[:, 2 * g:2 * g + 2, K:2 * K + 1],
                            start=(g == 0), stop=(g == 1), perf_mode=DR,
                        )
                    if ib == 0:
                        nc.vector.tensor_copy(vtz_acc[h][:], pz[:])
                    else:
                        nc.vector.tensor_add(vtz_acc[h][:], vtz_acc[h][:], pz[:])

            # ---- AllReduce the [k,k+1] + csU partials ----
            csu = [accp.tile([P, 1], f32, tag=f"csu{h}", name=f"csu{h}") for h in range(2)]
            for h in range(2):
                nc.vector.reduce_sum(csu[h][:], csu_cols[h][:], axis=mybir.AxisListType.X)
            bin_ = dram.tile([2 * P + 2, K + 1], f32)
            bout = dram.tile([2 * P + 2, K + 1], f32)
            for h in range(2):
                nc.sync.dma_start(out=bin_[h * P:(h + 1) * P, :], in_=vtz_acc[h][:])
                nc.sync.dma_start(
                    out=bin_[2 * P + h, 0:P].rearrange("(p one) -> p one", one=1),
                    in_=csu[h][:],
                )
            nc.gpsimd.collective_compute(
                "AllReduce", mybir.AluOpType.add,
                replica_groups=[list(range(NCORES))],
                ins=[bin_.opt()], outs=[bout.opt()],
            )

            # ---- phase B: bf16 T^T pass (overlaps the AllReduce) ----
            for ib in range(NB):
                i0 = ib * IB
                xbt = xbp.tile([P, dj, IB], bf16, tag="xbt")
                nc.sync.dma_start(out=xbt[:], in_=xbd[:, :, i0:i0 + IB])
                for h in range(2):
                    pt = ps.tile([P, IB], f32, tag="work")
                    for kd in range(dj):
                        nc.tensor.matmul(
                            pt[:], wtt[:, kd, h * P:(h + 1) * P], xbt[:, kd, :],
                            start=(kd == 0), stop=(kd == dj - 1),
                        )
                    tt = op.tile([P, IB], f32, tag="tt")
                    nc.scalar.activation(tt[:], pt[:], RELU)
                    nc.scalar.dma_start(out=outT[h * P:(h + 1) * P, i0:i0 + IB], in_=tt[:])

            # ---- phase C: D = 1/(csU.csV/(SW^2 n) + eps); res^T = (M8^T U8^T)/(SW*S4) ----
            vtzf = [accp.tile([P, K + 1], f32, tag=f"vtzf{h}", name=f"vtzf{h}")
                    for h in range(2)]
            for h in range(2):
                nc.sync.dma_start(out=vtzf[h][:], in_=bout[h * P:(h + 1) * P, :])
            csut = accp.tile([P, 2], f32, tag="csut")
            nc.sync.dma_start(
                out=csut[:], in_=bout[2 * P:2 * P + 2, 0:P].rearrange("t p -> p t")
            )
            csvt = accp.tile([P, 2], f32, tag="csvt")
            for h in range(2):
                nc.vector.tensor_copy(csvt[:, h:h + 1], vtzf[h][:, K:K + 1])
            pdot = ps.tile([1, 1], f32, tag="work")
            for h in range(2):
                nc.tensor.matmul(
                    pdot[:], csut[:, h:h + 1], csvt[:, h:h + 1],
                    start=(h == 0), stop=(h == 1),
                )
            dsb = accp.tile([1, 1], f32, tag="dsb")
            nc.vector.tensor_scalar(
                out=dsb[:], in0=pdot[:], scalar1=1.0 / (SW * SW * N_ROWS), scalar2=EPS,
                op0=mybir.AluOpType.mult, op1=mybir.AluOpType.add,
            )
            nc.vector.reciprocal(dsb[:], dsb[:])
            pb = ps.tile([P, 1], f32, tag="work")
            nc.tensor.matmul(pb[:], ones_row[:], dsb[:], start=True, stop=True)
            dbc = accp.tile([P, 1], f32, tag="dbc")
            nc.vector.tensor_copy(dbc[:], pb[:])
            # M8 = fp8(vtz_acc_allreduced * D * S4/SW^2); S4 == SW^2 so scale is D
            m8 = accp.tile([P, 2, K], f8, tag="m8")
            for h in range(2):
                nc.vector.tensor_scalar_mul(m8[:, h, :], vtzf[h][:, 0:K], dbc[:])
            for ib in range(NB):
                i0 = ib * IB
                for mc in range(2):
                    pr = ps.tile([P, IB], f32, tag="work")
                    nc.tensor.matmul(
                        pr[:], m8[:, :, mc * P:(mc + 1) * P], ut[:, :, i0:i0 + IB],
                        start=True, stop=True, perf_mode=DR,
                    )
                    rt = op.tile([P, IB], f32, tag="tt")
                    nc.scalar.activation(rt[:], pr[:], COPY, scale=1.0 / (SW * S4))
                    nc.scalar.dma_start(out=outR[mc * P:(mc + 1) * P, i0:i0 + IB], in_=rt[:])

    nc.compile()
    return nc


def _get_nc(dj):
    if dj not in _built:
        _built[dj] = _build(dj)
    return _built[dj]


def _pack(arrT, dj, dtype):
    """arrT: [d_rows, m] (d_rows <= dj*128, zero-padded) -> [128, dj, m]."""
    import ml_dtypes  # noqa: F401
    d_rows, m = arrT.shape
    if d_rows < dj * P:
        pad = np.zeros((dj * P, m), np.float32)
        pad[:d_rows] = arrT
        arrT = pad
    return np.ascontiguousarray(
        arrT.reshape(dj, P, m).transpose(1, 0, 2)
    ).astype(dtype)


def _run(x, W, b, trace=False, trace_cores=None):
    import ml_dtypes
    from concourse.bass_utils import run_bass_kernel_spmd

    f8np = ml_dtypes.float8_e4m3
    bfnp = ml_dtypes.bfloat16
    x = np.ascontiguousarray(x, dtype=np.float32)
    W = np.asarray(W, dtype=np.float32)
    b = np.asarray(b, dtype=np.float32)
    if np.any(b):
        dj = 10                 # pad contraction: ones-row in x picks up b from W
        w_uvz = np.concatenate([W[:3 * K].T * SW, (b[:3 * K] * SW)[None, :]], axis=0)
        w_t = np.concatenate([W[3 * K:].T, b[3 * K:][None, :]], axis=0)
    else:
        dj = D_IN // P
        w_uvz = W[:3 * K].T * SW
        w_t = W[3 * K:].T
    nc = _get_nc(dj)
    w8 = _pack(w_uvz, dj, f8np)
    wt = _pack(w_t, dj, bfnp)
    in_maps = []
    for c in range(NCORES):
        xsT = x[c * NLOC:(c + 1) * NLOC].T
        if dj * P > D_IN:
            xsT = np.concatenate(
                [xsT, np.ones((1, NLOC), np.float32)], axis=0)
        xsTp = np.ascontiguousarray(
            np.concatenate([xsT, np.zeros((dj * P - xsT.shape[0], NLOC), np.float32)])
            if xsT.shape[0] < dj * P else xsT
        ).reshape(dj, P, NLOC).transpose(1, 0, 2)
        in_maps.append({
            "x8": np.ascontiguousarray(xsTp).astype(f8np),
            "xb": np.ascontiguousarray(xsTp).astype(bfnp),
            "w8": w8, "wt": wt,
        })
    res = run_bass_kernel_spmd(
        nc, in_maps, list(range(NCORES)),
        trace=trace, **({"trace_cores": trace_cores} if trace_cores else {}),
    )
    full = np.empty((N_ROWS, 2 * K), np.float32)
    for c in range(NCORES):
        full[c * NLOC:(c + 1) * NLOC, 0:K] = res.results[c]["outR"].T
        full[c * NLOC:(c + 1) * NLOC, K:2 * K] = res.results[c]["outT"].T
    return full, res


def kernel(x, W, b):
    full, _ = _run(x, W, b)
    return full


# revision 8
# speedup vs baseline: 1.6452x; 1.1954x over previous
"""Low-rank attention Trainium2 kernel (8 NeuronCores, SPMD) — fp8 DoubleRow.

Math (reference):
    tmp = relu(x @ W.T + b); U,V,Z,T = split(tmp, 4, axis=1)
    norm = sum(U @ colsum(V)) / n + eps ;  D = 1/norm
    out = concat[(U @ (V.T @ Z)) * D, T]

Sharding: rows of x across 8 cores. Per-core partials (V.T@Z [k,k],
colsum(V), colsum(U)) are AllReduced on-device; each core then computes
its local U @ (VtZ) * D.

Precision strategy (rel-err budget 2e-2; this lands ~4e-3):
  - U/V/Z projection, VtZ, colsums and U@(VtZ) run in fp8e4m3 with the
    tensor engine's DoubleRow perf mode (2 fp8 MACs/PE/cycle, paired
    256-deep contraction per instruction). Elementwise fp8 noise washes
    out in the n=65536 (VtZ/colsum) and k=256 (U@VtZ) reductions.
  - T passthrough is bf16 (error shows up directly in the output).
  - The AllReduce payload and both outputs are bf16; W_uvz is pre-scaled
    by SW=16 to clear the fp8 subnormal range; scales fold into the
    final copies (exact powers of two).

Schedule: phase A streams x(fp8) once for U^T / V|Z / VtZ partials;
the SBUF-resident bf16 AllReduce then overlaps phase B (bf16 T^T pass
re-streaming x as bf16); phase C applies U @ (VtZ*D). T and res are
computed transposed so every matmul keeps a 512-wide moving dim; the
host transposes them back during the gather.
"""
import sys

sys.path.insert(0, "/opt/trn_rl_repo")
import numpy as np

NCORES = 8
N_ROWS, D_IN, K = 65536, 1024, 256
NLOC = N_ROWS // NCORES      # 8192 rows per core
P = 128
IB = 512                     # i-block width
NB = NLOC // IB              # 16 blocks
EPS = 1e-6
SW = 16.0                    # fp8 weight pre-scale
S4 = 256.0                   # VtZ*D quantization scale
NPRE = 4                     # xb blocks prefetched during phase A

_built = {}


def _build(dj):
    """dj = number of 128-row contraction chunks (8 normally, 10 with bias pad)."""
    import concourse.bacc as bacc
    import concourse.mybir as mybir
    import concourse.tile as tile

    dt = mybir.dt
    f32, f8, bf16 = dt.float32, dt.float8e4, dt.bfloat16
    RELU = mybir.ActivationFunctionType.Relu
    COPY = mybir.ActivationFunctionType.Copy
    DR = mybir.MatmulPerfMode.DoubleRow
    NJP = dj // 2            # DoubleRow contraction pair count

    nc = bacc.Bacc("TRN2", target_bir_lowering=False, debug=False, num_devices=NCORES)
    x8d = nc.dram_tensor("x8", [P, dj, NLOC], f8, kind="ExternalInput")
    xbd = nc.dram_tensor("xb", [P, dj, NLOC], bf16, kind="ExternalInput")
    w8d = nc.dram_tensor("w8", [P, dj, 3 * K], f8, kind="ExternalInput")
    wtd = nc.dram_tensor("wt", [P, dj, K], bf16, kind="ExternalInput")
    outR = nc.dram_tensor("outR", [K, NLOC], bf16, kind="ExternalOutput")
    outT = nc.dram_tensor("outT", [K, NLOC], bf16, kind="ExternalOutput")

    with tile.TileContext(nc) as tc:
        with (
            tc.tile_pool(name="wp", bufs=1) as wp,
            tc.tile_pool(name="xp", bufs=3) as xp,
            tc.tile_pool(name="xbp", bufs=NPRE) as xbp,
            tc.tile_pool(name="up", bufs=1) as up,
            tc.tile_pool(name="vzp", bufs=2) as vzp,
            tc.tile_pool(name="op", bufs=4) as op,
            tc.tile_pool(name="acc", bufs=1) as accp,
            tc.tile_pool(name="ps", bufs=6, space="PSUM") as ps,
            tc.tile_pool(name="vps", bufs=2, space="PSUM") as vps,
            tc.tile_pool(name="dram", bufs=1, space="DRAM") as dram,
        ):
            # W preload split across three DMA queues; U-part first on sync
            # so block-0 U matmuls can start early.
            w8t = wp.tile([P, dj, 3 * K], f8, tag="w8t")
            nc.sync.dma_start(out=w8t[:, :, 0:K], in_=w8d[:, :, 0:K])
            nc.scalar.dma_start(out=w8t[:, :, K:2 * K], in_=w8d[:, :, K:2 * K])
            nc.gpsimd.dma_start(out=w8t[:, :, 2 * K:3 * K], in_=w8d[:, :, 2 * K:3 * K])
            wtt = wp.tile([P, dj, K], bf16, tag="wtt")
            nc.gpsimd.dma_start(out=wtt[:], in_=wtd[:, :, :])
            ones_row = wp.tile([1, P], f32, tag="ones_row")
            nc.vector.memset(ones_row[:], 1.0)

            ut = up.tile([P, 2, NLOC], f8, tag="ut")
            csu_cols = [accp.tile([P, NB], f32, tag=f"csuc{h}", name=f"csuc{h}")
                        for h in range(2)]
            vtz_acc = [accp.tile([P, K + 1], f32, tag=f"vtza{h}", name=f"vtza{h}")
                       for h in range(2)]

            # ---- phase A: fp8 projection (U^T, V|Z) + VtZ/colsum partials ----
            xbt_pre = {}
            for ib in range(NB):
                i0 = ib * IB
                x8t = xp.tile([P, dj, IB], f8, tag="x8t")
                nc.sync.dma_start(out=x8t[:], in_=x8d[:, :, i0:i0 + IB])
                # U^T [k-chunk, i]: stationary W pairs, moving x pairs
                for h in range(2):
                    pu = ps.tile([P, IB], f32, tag="work")
                    for jp in range(NJP):
                        nc.tensor.matmul(
                            pu[:], w8t[:, 2 * jp:2 * jp + 2, h * P:(h + 1) * P],
                            x8t[:, 2 * jp:2 * jp + 2, :],
                            start=(jp == 0), stop=(jp == NJP - 1), perf_mode=DR,
                        )
                    nc.scalar.activation(
                        ut[:, h, i0:i0 + IB], pu[:], RELU,
                        accum_out=csu_cols[h][:, ib:ib + 1],
                    )
                # V|Z natural [i, col] per 128-row subtile; ones col at 2K
                vzt = vzp.tile([P, 4, 3 * K], f8, tag="vzt")
                nc.vector.memset(vzt[:, :, 2 * K:2 * K + 1], 1.0)
                for s in range(4):
                    pvz = ps.tile([P, 2 * K], f32, tag="work")
                    for jp in range(NJP):
                        nc.tensor.matmul(
                            pvz[:], x8t[:, 2 * jp:2 * jp + 2, s * P:(s + 1) * P],
                            w8t[:, 2 * jp:2 * jp + 2, K:3 * K],
                            start=(jp == 0), stop=(jp == NJP - 1), perf_mode=DR,
                        )
                    nc.vector.tensor_relu(vzt[:, s, 0:2 * K], pvz[:])
                # VtZ partial + colsum(V) via the ones column: V^T @ [Z | 1]
                for h in range(2):
                    pz = vps.tile([P, K + 1], f32, tag="vtzw")
                    for g in range(2):
                        nc.tensor.matmul(
                            pz[:], vzt[:, 2 * g:2 * g + 2, h * P:(h + 1) * P],
                            vzt[:, 2 * g:2 * g + 2, K:2 * K + 1],
                            start=(g == 0), stop=(g == 1), perf_mode=DR,
                        )
                    if ib == 0:
                        nc.vector.tensor_copy(vtz_acc[h][:], pz[:])
                    else:
                        nc.vector.tensor_add(vtz_acc[h][:], vtz_acc[h][:], pz[:])
                # prefetch first xb blocks so phase B starts fed
                if ib >= NB - NPRE:
                    pb_ = ib - (NB - NPRE)
                    xbt = xbp.tile([P, dj, IB], bf16, tag="xbt", name=f"xbtp{pb_}")
                    nc.sync.dma_start(out=xbt[:], in_=xbd[:, :, pb_ * IB:(pb_ + 1) * IB])
                    xbt_pre[pb_] = xbt

            # ---- SBUF-resident bf16 AllReduce of [k,k]+csV+csU partials ----
            csu = [accp.tile([P, 1], f32, tag=f"csu{h}", name=f"csu{h}") for h in range(2)]
            for h in range(2):
                nc.vector.reduce_sum(csu[h][:], csu_cols[h][:], axis=mybir.AxisListType.X)
            arin = accp.tile([P, 2 * K + 4], bf16, tag="arin")
            arout = accp.tile([P, 2 * K + 4], bf16, tag="arout")
            for h in range(2):
                nc.vector.tensor_copy(arin[:, h * K:(h + 1) * K], vtz_acc[h][:, 0:K])
                nc.vector.tensor_copy(arin[:, 2 * K + h:2 * K + h + 1], vtz_acc[h][:, K:K + 1])
                nc.vector.tensor_copy(arin[:, 2 * K + 2 + h:2 * K + 3 + h], csu[h][:])
            bin_ = dram.tile([P, 2 * K + 4], bf16)
            bout = dram.tile([P, 2 * K + 4], bf16)
            nc.scalar.dma_start(out=bin_[:, :], in_=arin[:])
            nc.gpsimd.collective_compute(
                "AllReduce", mybir.AluOpType.add,
                replica_groups=[list(range(NCORES))],
                ins=[bin_.opt()], outs=[bout.opt()],
            )
            nc.scalar.dma_start(out=arout[:], in_=bout[:, :])

            # ---- phase B: bf16 T^T pass (overlaps the AllReduce) ----
            for ib in range(NB):
                i0 = ib * IB
                if ib in xbt_pre:
                    xbt = xbt_pre.pop(ib)
                else:
                    xbt = xbp.tile([P, dj, IB], bf16, tag="xbt", name=f"xbt{ib}")
                    nc.sync.dma_start(out=xbt[:], in_=xbd[:, :, i0:i0 + IB])
                for h in range(2):
                    pt = ps.tile([P, IB], f32, tag="work")
                    for kd in range(dj):
                        nc.tensor.matmul(
                            pt[:], wtt[:, kd, h * P:(h + 1) * P], xbt[:, kd, :],
                            start=(kd == 0), stop=(kd == dj - 1),
                        )
                    tt = op.tile([P, IB], bf16, tag="tt")
                    if h == 0:
                        nc.scalar.activation(tt[:], pt[:], RELU)
                    else:
                        nc.vector.tensor_relu(tt[:], pt[:])
                    nc.sync.dma_start(out=outT[h * P:(h + 1) * P, i0:i0 + IB], in_=tt[:])

            # ---- phase C: D = 1/(csU.csV/(SW^2 n) + eps); res^T = (M8^T U8^T)/(SW*S4) ----
            pdot = ps.tile([1, 1], f32, tag="work")
            for h in range(2):
                nc.tensor.matmul(
                    pdot[:], arout[:, 2 * K + 2 + h:2 * K + 3 + h],
                    arout[:, 2 * K + h:2 * K + h + 1],
                    start=(h == 0), stop=(h == 1),
                )
            dsb = accp.tile([1, 1], f32, tag="dsb")
            nc.vector.tensor_scalar(
                out=dsb[:], in0=pdot[:], scalar1=1.0 / (SW * SW * N_ROWS), scalar2=EPS,
                op0=mybir.AluOpType.mult, op1=mybir.AluOpType.add,
            )
            nc.vector.reciprocal(dsb[:], dsb[:])
            pb = ps.tile([P, 1], f32, tag="work")
            nc.tensor.matmul(pb[:], ones_row[:], dsb[:], start=True, stop=True)
            dbc = accp.tile([P, 1], f32, tag="dbc")
            nc.vector.tensor_copy(dbc[:], pb[:])
            # M8 = fp8(vtz_allreduced * D * S4/SW^2); S4 == SW^2 so scale is D
            m8 = accp.tile([P, 2, K], f8, tag="m8")
            for h in range(2):
                nc.vector.tensor_scalar_mul(m8[:, h, :], arout[:, h * K:(h + 1) * K], dbc[:])
            for ib in range(NB):
                i0 = ib * IB
                for mc in range(2):
                    pr = ps.tile([P, IB], f32, tag="work")
                    nc.tensor.matmul(
                        pr[:], m8[:, :, mc * P:(mc + 1) * P], ut[:, :, i0:i0 + IB],
                        start=True, stop=True, perf_mode=DR,
                    )
                    rt = op.tile([P, IB], bf16, tag="tt")
                    if mc == 0:
                        nc.scalar.activation(rt[:], pr[:], COPY, scale=1.0 / (SW * S4))
                    else:
                        nc.vector.tensor_scalar_mul(rt[:], pr[:], 1.0 / (SW * S4))
                    nc.sync.dma_start(out=outR[mc * P:(mc + 1) * P, i0:i0 + IB], in_=rt[:])

    nc.compile()
    return nc


def _get_nc(dj):
    if dj not in _built:
        _built[dj] = _build(dj)
    return _built[dj]


def _pack(arrT, dj, dtype):
    """arrT: [d_rows, m] (d_rows <= dj*128, zero-padded) -> [128, dj, m]."""
    d_rows, m = arrT.shape
    if d_rows < dj * P:
        pad = np.zeros((dj * P, m), np.float32)
        pad[:d_rows] = arrT
        arrT = pad
    return np.ascontiguousarray(
        arrT.reshape(dj, P, m).transpose(1, 0, 2)
    ).astype(dtype)


def _run(x, W, b, trace=False, trace_cores=None):
    import ml_dtypes
    from concourse.bass_utils import run_bass_kernel_spmd

    f8np = ml_dtypes.float8_e4m3
    bfnp = ml_dtypes.bfloat16
    x = np.ascontiguousarray(x, dtype=np.float32)
    W = np.asarray(W, dtype=np.float32)
    b = np.asarray(b, dtype=np.float32)
    if np.any(b):
        dj = 10                 # pad contraction: ones-row in x picks up b from W
        w_uvz = np.concatenate([W[:3 * K].T * SW, (b[:3 * K] * SW)[None, :]], axis=0)
        w_t = np.concatenate([W[3 * K:].T, b[3 * K:][None, :]], axis=0)
    else:
        dj = D_IN // P
        w_uvz = W[:3 * K].T * SW
        w_t = W[3 * K:].T
    nc = _get_nc(dj)
    w8 = _pack(w_uvz, dj, f8np)
    wt = _pack(w_t, dj, bfnp)
    in_maps = []
    for c in range(NCORES):
        xsT = x[c * NLOC:(c + 1) * NLOC].T
        if dj * P > D_IN:
            xsT = np.concatenate(
                [xsT, np.ones((1, NLOC), np.float32)], axis=0)
        if xsT.shape[0] < dj * P:
            xsT = np.concatenate(
                [xsT, np.zeros((dj * P - xsT.shape[0], NLOC), np.float32)])
        xsTp = np.ascontiguousarray(
            np.ascontiguousarray(xsT).reshape(dj, P, NLOC).transpose(1, 0, 2)
        )
        in_maps.append({
            "x8": xsTp.astype(f8np),
            "xb": xsTp.astype(bfnp),
            "w8": w8, "wt": wt,
        })
    res = run_bass_kernel_spmd(
        nc, in_maps, list(range(NCORES)),
        trace=trace, **({"trace_cores": trace_cores} if trace_cores else {}),
    )
    full = np.empty((N_ROWS, 2 * K), np.float32)
    for c in range(NCORES):
        full[c * NLOC:(c + 1) * NLOC, 0:K] = res.results[c]["outR"].T.astype(np.float32)
        full[c * NLOC:(c + 1) * NLOC, K:2 * K] = res.results[c]["outT"].T.astype(np.float32)
    return full, res


def kernel(x, W, b):
    full, _ = _run(x, W, b)
    return full
